# revision 16
# baseline (speedup 1.0000x reference)
"""VQ codebook (k-means, 10 epochs) Trainium2 kernel — float32r rewrite.

Problem: patches [40000, 4, 16, 5, 5] f32, centroids_init [4, 64, 400] f32.
Per epoch (x10): scores = c@p^T - 0.5||c||^2, labels = argmax, one-hot
summation + counts, new centroids = sum/counts (zero empty clusters).

Strategy (8 NeuronCores, data parallel over patches; n_local=5000/core):
  All heavy matmuls use float32r (replicated-fp32 PE mode): 1 cycle/row
  when the moving free dim >= 256, vs 4 cycles/row for plain fp32.
  - scores^T [64, nb] per (group, block<=512): centroids stationary
    [101, 64] (row 100 = -0.5||c||^2 bias), patches stream [101, nb]
    (row 100 = ones). 4 d-chunks accumulate in PSUM.
  - PE-transpose scores^T -> [nj<=128, 64] per n-chunk; DVE free-axis
    reduce_max + is_equal gives one-hot S [nj, 64] in SBUF directly.
  - patches arrive in ONE layout (pf_dn [G, D, n]); the n-on-partitions
    layout for the summation is built on-chip: PE-transpose [100, nj] ->
    [nj, 100] x4 chunks -> scalar-engine copy -> pfnd [nj, 401] (col 400
    = ones for counts). Halves HBM traffic vs shipping both layouts.
  - summation: S stationary [nj, 64], pfnd streams [nj, 401] -> psum
    [64, 401] accumulated over all 40 n-chunks of the epoch (4 banks).
  - per epoch: AllReduce [4, 64, 401] across 8 cores; replicated update
    (sums/counts, zero empty clusters) + rebuild of centroid chunks.
"""

import sys

sys.path.insert(0, "/opt/trn_rl_repo")

import numpy as np
from contextlib import ExitStack

import concourse.bass as bass
import concourse.bacc as bacc
import concourse.tile as tile
from concourse import mybir
from concourse import bass_utils

G = 4
K = 64
D = 400
CH = 100           # contraction chunk (d)
NCH = D // CH      # 4 chunks
NB = 512           # patch block (moving free dim)
NJ = 128           # n-chunk (summation contraction)
F32 = mybir.dt.float32
F32R = mybir.dt.float32r
BF16 = mybir.dt.bfloat16
AX = mybir.AxisListType
ALU = mybir.AluOpType
NSUM = D + 2       # even moving dim for the f32r summation matmul

MM_DT = F32R       # dtype tag for the big matmuls
TR_DT = F32        # dtype tag for PE transposes


def _r(ap):
    """bitcast an f32 AP to the matmul dtype."""
    return ap.bitcast(MM_DT) if MM_DT is not F32 else ap


def _t(ap):
    return ap.bitcast(TR_DT) if TR_DT is not F32 else ap


def build_nc(n_local: int, epochs: int, n_cores: int):
    blocks = []
    n0 = 0
    while n0 < n_local:
        blocks.append((n0, min(NB, n_local - n0)))
        n0 += NB

    nc = bacc.Bacc("TRN2", target_bir_lowering=False, debug=False,
                   num_devices=n_cores)

    pf_dn = nc.dram_tensor("pf_dn", [G, D, n_local], F32,
                           kind="ExternalInput").ap()
    pf_nd = nc.dram_tensor("pf_nd", [n_local, G * 2 * NSUM], BF16,
                           kind="ExternalInput").ap()
    cent = nc.dram_tensor("cent", [G, K, D], F32, kind="ExternalInput").ap()
    out = nc.dram_tensor("out", [G, K, D], F32, kind="ExternalOutput").ap()

    ident_dram = nc.inline_tensor(np.eye(128, dtype=np.float32), name="ident")

    with tile.TileContext(nc) as tc, ExitStack() as ctx:
        pool_const = ctx.enter_context(tc.tile_pool(name="const", bufs=1))
        pool_pft = ctx.enter_context(tc.tile_pool(name="pft", bufs=2))
        pool_pfnd = ctx.enter_context(tc.tile_pool(name="pfnd", bufs=2))
        pool_scb = ctx.enter_context(tc.tile_pool(name="scb", bufs=2))
        pool_s = ctx.enter_context(tc.tile_pool(name="s", bufs=2))
        pool_m = ctx.enter_context(tc.tile_pool(name="m", bufs=2))
        pool_upd = ctx.enter_context(tc.tile_pool(name="upd", bufs=1))
        # PSUM budget (8 banks): sums 4 + scores 2 + scT 2
        pool_ps_sum = ctx.enter_context(
            tc.tile_pool(name="ps_sum", bufs=1, space="PSUM"))
        pool_ps_sc = ctx.enter_context(
            tc.tile_pool(name="ps_sc", bufs=2, space="PSUM"))
        pool_ps_sct = ctx.enter_context(
            tc.tile_pool(name="ps_sct", bufs=2, space="PSUM"))
        pool_dram = ctx.enter_context(
            tc.tile_pool(name="dram", bufs=1, space="DRAM"))

        ident = pool_const.tile([128, 128], F32, tag="ident")
        nc.sync.dma_start(ident[:], ident_dram.ap()[:, :])

        # persistent sbuf state
        ct_sb = pool_const.tile([CH, G * NCH * K], F32, tag="ct")
        newc = pool_const.tile([K, G * D], F32, tag="newc")       # [64, 1600]
        sums_sb = pool_const.tile([K, G * (D + 1)], F32, tag="sums_sb")
        red_sb = pool_const.tile([K, G * (D + 1)], F32, tag="red_sb")
        sq = pool_upd.tile([K, D], F32, tag="sq")
        c2n = pool_upd.tile([K, G], F32, tag="c2n")
        cnt_all = pool_upd.tile([K, G], F32, tag="cnt")
        dv = pool_upd.tile([K, G], F32, tag="dv")
        minc = pool_upd.tile([K, 1], F32, tag="minc")
        alive = pool_upd.tile([K, 1], F32, tag="alive")

        bounce_in = pool_dram.tile([G, K, D + 1], F32, tag="bin")
        bounce_out = pool_dram.tile([G, K, D + 1], F32, tag="bout")

        def rebuild_ct():
            """newc [64, g*400+:400] -> ct_sb chunks [100, 64]; c2n bias."""
            for g in range(G):
                nc.vector.tensor_mul(sq[:, :], newc[:, g * D:(g + 1) * D],
                                     newc[:, g * D:(g + 1) * D])
                nc.vector.reduce_sum(c2n[:, g:g + 1], sq[:, :], axis=AX.X)
                nc.vector.tensor_scalar_mul(c2n[:, g:g + 1], c2n[:, g:g + 1],
                                            -0.5)
            for g in range(G):
                ps = pool_ps_sct.tile([128, NCH * K], F32, tag="sct")
                for c in range(NCH):
                    nc.tensor.transpose(
                        ps[0:CH, c * K:(c + 1) * K],
                        newc[:, g * D + c * CH:g * D + (c + 1) * CH],
                        ident[0:K, 0:K])
                nc.vector.tensor_copy(
                    ct_sb[0:CH, g * NCH * K:(g + 1) * NCH * K],
                    ps[0:CH, 0:NCH * K])

        # init: load centroids_init into newc layout, build ct
        for g in range(G):
            nc.sync.dma_start(newc[:, g * D:(g + 1) * D], cent[g, :, :])
        rebuild_ct()

        for ep in range(epochs):
            sum_ps = [pool_ps_sum.tile([K, NSUM], F32, tag=f"sum{g}",
                                       name=f"sum{g}")
                      for g in range(G)]
            first_mm = [True] * G
            n_done = 0

            for (n0, nb) in blocks:
                last_blk = (n0 + nb >= n_local)
                njs = []
                j0 = 0
                while j0 < nb:
                    njs.append((j0, min(NJ, nb - j0)))
                    j0 += NJ

                pft = {}
                for g in range(G):
                    for c in range(NCH):
                        t = pool_pft.tile([CH, NB], F32,
                                          tag=f"pft{g}_{c}")
                        nc.sync.dma_start(
                            t[0:CH, 0:nb],
                            pf_dn[g, c * CH:(c + 1) * CH, n0:n0 + nb])
                        pft[(g, c)] = t
                pfnd = {}
                for j, (j0, nj) in enumerate(njs):
                    pt = pool_pfnd.tile([NJ, G * 2 * NSUM], BF16,
                                        tag=f"pfnd{j}")
                    nc.sync.dma_start(pt[0:nj, :],
                                      pf_nd[n0 + j0:n0 + j0 + nj, :])
                    pfnd[j] = pt

                for g in range(G):
                    # ---- scores^T [64, nb] (bias added in the copy) ----
                    sc = pool_ps_sc.tile([K, NB], F32, tag="sc")
                    for c in range(NCH):
                        nc.tensor.matmul(
                            sc[:, 0:nb],
                            lhsT=ct_sb[0:CH, (g * NCH + c) * K:
                                       (g * NCH + c + 1) * K],
                            rhs=pft[(g, c)][0:CH, 0:nb],
                            start=(c == 0), stop=(c == NCH - 1))
                    scb = pool_scb.tile([K, NB], F32, tag=f"scb{g}")
                    nc.vector.tensor_scalar_add(scb[:, 0:nb], sc[:, 0:nb],
                                                c2n[:, g:g + 1])

                    # ---- transpose to [nj, 64] + one-hot ----
                    sct = pool_ps_sct.tile([128, NCH * K], F32, tag="sct")
                    for j, (j0, nj) in enumerate(njs):
                        nc.tensor.transpose(
                            sct[0:nj, j * K:(j + 1) * K],
                            _t(scb[0:K, j0:j0 + nj]),
                            _t(ident[0:K, 0:K]))
                    nch = len(njs)
                    m = pool_m.tile([NJ, NCH], F32, tag=f"m{g}")
                    nc.vector.reduce_max(
                        m[:, 0:nch],
                        sct[:, 0:nch * K].rearrange(
                            "p (j k) -> p j k", j=nch),
                        axis=AX.X)
                    sbig = pool_s.tile([NJ, NCH * K], BF16, tag=f"s{g}")
                    nc.vector.tensor_tensor(
                        out=sbig[:, 0:nch * K].rearrange(
                            "p (j k) -> p j k", j=nch),
                        in0=sct[:, 0:nch * K].rearrange(
                            "p (j k) -> p j k", j=nch),
                        in1=m[:, 0:nch].rearrange(
                            "p j -> p j ()").to_broadcast((NJ, nch, K)),
                        op=ALU.is_equal)

                    # ---- summation: S stationary, bf16 hi/lo streams ----
                    for j, (j0, nj) in enumerate(njs):
                        for h in range(2):
                            nc.tensor.matmul(
                                sum_ps[g][:, :],
                                lhsT=sbig[0:nj, j * K:(j + 1) * K],
                                rhs=pfnd[j][0:nj,
                                            (g * 2 + h) * NSUM:
                                            (g * 2 + h + 1) * NSUM],
                                start=(first_mm[g] and h == 0),
                                stop=(last_blk and j == len(njs) - 1
                                      and h == 1),
                                skip_group_check=True)
                        first_mm[g] = False
                n_done += nb

            # ---- epoch boundary: allreduce + update ----
            for g in range(G):
                nc.scalar.copy(
                    sums_sb[:, g * (D + 1):(g + 1) * (D + 1)],
                    sum_ps[g][:, 0:D + 1])
                nc.sync.dma_start(
                    bounce_in[g, :, :],
                    sums_sb[:, g * (D + 1):(g + 1) * (D + 1)])
                if n_cores > 1:
                    nc.gpsimd.collective_compute(
                        "AllReduce", ALU.add,
                        replica_groups=[list(range(n_cores))],
                        ins=[bounce_in[g, :, :].opt()],
                        outs=[bounce_out[g, :, :].opt()])
                else:
                    nc.sync.dma_start(bounce_out[g, :, :],
                                      bounce_in[g, :, :])
                nc.sync.dma_start(
                    red_sb[:, g * (D + 1):(g + 1) * (D + 1)],
                    bounce_out[g, :, :])
                cnt = red_sb[:, g * (D + 1) + D:g * (D + 1) + D + 1]
                nc.vector.tensor_copy(cnt_all[:, g:g + 1], cnt)
                nc.vector.tensor_scalar_max(dv[:, g:g + 1], cnt, 1.0)
                nc.vector.reciprocal(dv[:, g:g + 1], dv[:, g:g + 1])
                nc.vector.tensor_scalar_mul(
                    newc[:, g * D:(g + 1) * D],
                    red_sb[:, g * (D + 1):g * (D + 1) + D],
                    dv[:, g:g + 1])
            nc.vector.tensor_reduce(minc[:, :], cnt_all[:, :], axis=AX.X,
                                    op=ALU.min)
            nc.vector.tensor_scalar(out=alive[:, :], in0=minc[:, :],
                                    scalar1=0.0, scalar2=None, op0=ALU.is_gt)
            for g in range(G):
                nc.vector.tensor_scalar_mul(newc[:, g * D:(g + 1) * D],
                                            newc[:, g * D:(g + 1) * D],
                                            alive[:, 0:1])
            if ep < epochs - 1:
                rebuild_ct()

        for g in range(G):
            nc.sync.dma_start(out[g, :, :], newc[:, g * D:(g + 1) * D])

    nc.compile()
    return nc


def shard_inputs(patches: np.ndarray, n_cores: int):
    """Full patches [N, G, C, H, W] -> per-core {pf_dn, pf_nd} arrays."""
    N = patches.shape[0]
    n_local = N // n_cores
    pf = np.ascontiguousarray(patches.reshape(N, G, D)).astype(np.float32,
                                                               copy=False)
    maps = []
    for c in range(n_cores):
        s = pf[c * n_local:(c + 1) * n_local]  # [n_local, G, D]
        import ml_dtypes
        bf = ml_dtypes.bfloat16
        aug = np.zeros((n_local, G, 2, NSUM), dtype=bf)
        hi = s.astype(bf)
        lo = (s - hi.astype(np.float32)).astype(bf)
        aug[:, :, 0, :D] = hi
        aug[:, :, 1, :D] = lo
        aug[:, :, 0, D:] = bf(1.0)
        maps.append({
            "pf_dn": np.ascontiguousarray(s.transpose(1, 2, 0)),
            "pf_nd": aug.reshape(n_local, G * 2 * NSUM),
        })
    return maps


_CACHE = {}


def kernel(patches: np.ndarray, centroids_init: np.ndarray) -> np.ndarray:
    patches = np.asarray(patches, dtype=np.float32)
    centroids_init = np.asarray(centroids_init, dtype=np.float32)
    N = patches.shape[0]
    n_cores = 8
    epochs = 10
    n_local = N // n_cores
    assert N % n_cores == 0

    key = (N, epochs, n_cores)
    if key not in _CACHE:
        _CACHE[key] = build_nc(n_local, epochs, n_cores)
    nc = _CACHE[key]

    in_maps = shard_inputs(patches, n_cores)
    for m in in_maps:
        m["cent"] = centroids_init

    res = bass_utils.run_bass_kernel_spmd(nc, in_maps,
                                          core_ids=list(range(n_cores)))
    c = res.results[0]["out"]
    C, H, W = 16, 5, 5
    return c.reshape(G * K, C, H, W).astype(np.float32)


if __name__ == "__main__":
    np.random.seed(0)
    p = np.random.randn(2000, G, 16, 5, 5).astype(np.float32)
    ci = (np.random.randn(G, K, D) * 0.1).astype(np.float32)
    print(kernel(p, ci).shape)


# revision 17
# speedup vs baseline: 1.1144x; 1.1144x over previous
"""VQ codebook (k-means, 10 epochs) Trainium2 kernel — float32r rewrite.

Problem: patches [40000, 4, 16, 5, 5] f32, centroids_init [4, 64, 400] f32.
Per epoch (x10): scores = c@p^T - 0.5||c||^2, labels = argmax, one-hot
summation + counts, new centroids = sum/counts (zero empty clusters).

Strategy (8 NeuronCores, data parallel over patches; n_local=5000/core):
  All heavy matmuls use float32r (replicated-fp32 PE mode): 1 cycle/row
  when the moving free dim >= 256, vs 4 cycles/row for plain fp32.
  - scores^T [64, nb] per (group, block<=512): centroids stationary
    [101, 64] (row 100 = -0.5||c||^2 bias), patches stream [101, nb]
    (row 100 = ones). 4 d-chunks accumulate in PSUM.
  - PE-transpose scores^T -> [nj<=128, 64] per n-chunk; DVE free-axis
    reduce_max + is_equal gives one-hot S [nj, 64] in SBUF directly.
  - patches arrive in ONE layout (pf_dn [G, D, n]); the n-on-partitions
    layout for the summation is built on-chip: PE-transpose [100, nj] ->
    [nj, 100] x4 chunks -> scalar-engine copy -> pfnd [nj, 401] (col 400
    = ones for counts). Halves HBM traffic vs shipping both layouts.
  - summation: S stationary [nj, 64], pfnd streams [nj, 401] -> psum
    [64, 401] accumulated over all 40 n-chunks of the epoch (4 banks).
  - per epoch: AllReduce [4, 64, 401] across 8 cores; replicated update
    (sums/counts, zero empty clusters) + rebuild of centroid chunks.
"""

import sys

sys.path.insert(0, "/opt/trn_rl_repo")

import numpy as np
from contextlib import ExitStack

import concourse.bass as bass
import concourse.bacc as bacc
import concourse.tile as tile
from concourse import mybir
from concourse import bass_utils

G = 4
K = 64
D = 400
CH = 100           # contraction chunk (d)
NCH = D // CH      # 4 chunks
NB = 512           # patch block (moving free dim)
NJ = 128           # n-chunk (summation contraction)
F32 = mybir.dt.float32
F32R = mybir.dt.float32r
BF16 = mybir.dt.bfloat16
AX = mybir.AxisListType
ALU = mybir.AluOpType
NSUM = D + 2       # even moving dim for the f32r summation matmul

MM_DT = F32R       # dtype tag for the big matmuls
TR_DT = F32        # dtype tag for PE transposes


def _r(ap):
    """bitcast an f32 AP to the matmul dtype."""
    return ap.bitcast(MM_DT) if MM_DT is not F32 else ap


def _t(ap):
    return ap.bitcast(TR_DT) if TR_DT is not F32 else ap


def build_nc(n_local: int, epochs: int, n_cores: int):
    blocks = []
    n0 = 0
    while n0 < n_local:
        blocks.append((n0, min(NB, n_local - n0)))
        n0 += NB

    nc = bacc.Bacc("TRN2", target_bir_lowering=False, debug=False,
                   num_devices=n_cores)

    pf_dn = nc.dram_tensor("pf_dn", [G, D, n_local], F32,
                           kind="ExternalInput").ap()
    pf_nd = nc.dram_tensor("pf_nd", [n_local, G * 2 * NSUM], BF16,
                           kind="ExternalInput").ap()
    cent = nc.dram_tensor("cent", [G, K, D], F32, kind="ExternalInput").ap()
    out = nc.dram_tensor("out", [G, K, D], F32, kind="ExternalOutput").ap()

    ident_dram = nc.inline_tensor(np.eye(128, dtype=np.float32), name="ident")

    with tile.TileContext(nc) as tc, ExitStack() as ctx:
        pool_const = ctx.enter_context(tc.tile_pool(name="const", bufs=1))
        pool_pft = ctx.enter_context(tc.tile_pool(name="pft", bufs=2))
        pool_pfnd = ctx.enter_context(tc.tile_pool(name="pfnd", bufs=2))
        pool_scb = ctx.enter_context(tc.tile_pool(name="scb", bufs=2))
        pool_s = ctx.enter_context(tc.tile_pool(name="s", bufs=2))
        pool_m = ctx.enter_context(tc.tile_pool(name="m", bufs=2))
        pool_upd = ctx.enter_context(tc.tile_pool(name="upd", bufs=1))
        # PSUM budget (8 banks): sums 4 + scores 2 + scT 2
        pool_ps_sum = ctx.enter_context(
            tc.tile_pool(name="ps_sum", bufs=1, space="PSUM"))
        pool_ps_sc = ctx.enter_context(
            tc.tile_pool(name="ps_sc", bufs=2, space="PSUM"))
        pool_ps_sct = ctx.enter_context(
            tc.tile_pool(name="ps_sct", bufs=2, space="PSUM"))
        pool_dram = ctx.enter_context(
            tc.tile_pool(name="dram", bufs=1, space="DRAM"))

        ident = pool_const.tile([128, 128], F32, tag="ident")
        nc.sync.dma_start(ident[:], ident_dram.ap()[:, :])

        # persistent sbuf state
        ct_sb = pool_const.tile([CH, G * NCH * K], F32, tag="ct")
        newc = pool_const.tile([K, G * D], F32, tag="newc")       # [64, 1600]
        sums_sb = pool_const.tile([K, G * (D + 1)], F32, tag="sums_sb")
        red_sb = pool_const.tile([K, G * (D + 1)], F32, tag="red_sb")
        sq = pool_upd.tile([K, D], F32, tag="sq")
        c2n = pool_upd.tile([K, G], F32, tag="c2n")
        cnt_all = pool_upd.tile([K, G], F32, tag="cnt")
        dv = pool_upd.tile([K, G], F32, tag="dv")
        minc = pool_upd.tile([K, 1], F32, tag="minc")
        alive = pool_upd.tile([K, 1], F32, tag="alive")

        bounce_in = pool_dram.tile([G, K, D + 1], F32, tag="bin")
        bounce_out = pool_dram.tile([G, K, D + 1], F32, tag="bout")

        def rebuild_ct():
            """newc [64, g*400+:400] -> ct_sb chunks [100, 64]; c2n bias."""
            for g in range(G):
                nc.vector.tensor_mul(sq[:, :], newc[:, g * D:(g + 1) * D],
                                     newc[:, g * D:(g + 1) * D])
                nc.vector.reduce_sum(c2n[:, g:g + 1], sq[:, :], axis=AX.X)
                nc.vector.tensor_scalar_mul(c2n[:, g:g + 1], c2n[:, g:g + 1],
                                            -0.5)
            for g in range(G):
                ps = pool_ps_sct.tile([128, NCH * K], F32, tag="sct")
                for c in range(NCH):
                    nc.tensor.transpose(
                        ps[0:CH, c * K:(c + 1) * K],
                        newc[:, g * D + c * CH:g * D + (c + 1) * CH],
                        ident[0:K, 0:K])
                nc.vector.tensor_copy(
                    ct_sb[0:CH, g * NCH * K:(g + 1) * NCH * K],
                    ps[0:CH, 0:NCH * K])

        # init: load centroids_init into newc layout, build ct
        for g in range(G):
            nc.sync.dma_start(newc[:, g * D:(g + 1) * D], cent[g, :, :])
        rebuild_ct()

        for ep in range(epochs):
            sum_ps = [pool_ps_sum.tile([K, NSUM], F32, tag=f"sum{g}",
                                       name=f"sum{g}")
                      for g in range(G)]
            first_mm = [True] * G
            n_done = 0

            for (n0, nb) in blocks:
                last_blk = (n0 + nb >= n_local)
                njs = []
                j0 = 0
                while j0 < nb:
                    njs.append((j0, min(NJ, nb - j0)))
                    j0 += NJ

                pft = {}
                for g in range(G):
                    for c in range(NCH):
                        t = pool_pft.tile([CH, NB], F32,
                                          tag=f"pft{g}_{c}")
                        nc.sync.dma_start(
                            t[0:CH, 0:nb],
                            pf_dn[g, c * CH:(c + 1) * CH, n0:n0 + nb])
                        pft[(g, c)] = t
                pfnd = {}
                for j, (j0, nj) in enumerate(njs):
                    pt = pool_pfnd.tile([NJ, G * 2 * NSUM], BF16,
                                        tag=f"pfnd{j}")
                    nc.sync.dma_start(pt[0:nj, :],
                                      pf_nd[n0 + j0:n0 + j0 + nj, :])
                    pfnd[j] = pt

                for g in range(G):
                    # ---- scores^T [64, nb] (bias added in the copy) ----
                    sc = pool_ps_sc.tile([K, NB], F32, tag="sc")
                    for c in range(NCH):
                        nc.tensor.matmul(
                            sc[:, 0:nb],
                            lhsT=ct_sb[0:CH, (g * NCH + c) * K:
                                       (g * NCH + c + 1) * K],
                            rhs=pft[(g, c)][0:CH, 0:nb],
                            start=(c == 0), stop=(c == NCH - 1))
                    scb = pool_scb.tile([K, NB], F32, tag=f"scb{g}")
                    nc.vector.tensor_scalar_add(scb[:, 0:nb], sc[:, 0:nb],
                                                c2n[:, g:g + 1])

                    # ---- transpose to [nj, 64] + one-hot ----
                    sct = pool_ps_sct.tile([128, NCH * K], F32, tag="sct")
                    for j, (j0, nj) in enumerate(njs):
                        nc.tensor.transpose(
                            sct[0:nj, j * K:(j + 1) * K],
                            _t(scb[0:K, j0:j0 + nj]),
                            _t(ident[0:K, 0:K]))
                    nch = len(njs)
                    m = pool_m.tile([NJ, NCH], F32, tag=f"m{g}")
                    nc.vector.reduce_max(
                        m[:, 0:nch],
                        sct[:, 0:nch * K].rearrange(
                            "p (j k) -> p j k", j=nch),
                        axis=AX.X)
                    sbig = pool_s.tile([NJ, NCH * K], BF16, tag=f"s{g}")
                    nc.vector.tensor_tensor(
                        out=sbig[:, 0:nch * K].rearrange(
                            "p (j k) -> p j k", j=nch),
                        in0=sct[:, 0:nch * K].rearrange(
                            "p (j k) -> p j k", j=nch),
                        in1=m[:, 0:nch].rearrange(
                            "p j -> p j ()").to_broadcast((NJ, nch, K)),
                        op=ALU.is_equal)

                    # ---- summation: S stationary, bf16 hi/lo streams ----
                    for j, (j0, nj) in enumerate(njs):
                        for h in range(2):
                            nc.tensor.matmul(
                                sum_ps[g][:, :],
                                lhsT=sbig[0:nj, j * K:(j + 1) * K],
                                rhs=pfnd[j][0:nj,
                                            (g * 2 + h) * NSUM:
                                            (g * 2 + h + 1) * NSUM],
                                start=(first_mm[g] and h == 0),
                                stop=(last_blk and j == len(njs) - 1
                                      and h == 1),
                                skip_group_check=True)
                        first_mm[g] = False
                n_done += nb

            # ---- epoch boundary: allreduce + update ----
            for g in range(G):
                nc.scalar.copy(
                    sums_sb[:, g * (D + 1):(g + 1) * (D + 1)],
                    sum_ps[g][:, 0:D + 1])
                nc.sync.dma_start(
                    bounce_in[g, :, :],
                    sums_sb[:, g * (D + 1):(g + 1) * (D + 1)])
            if n_cores > 1:
                nc.gpsimd.collective_compute(
                    "AllReduce", ALU.add,
                    replica_groups=[list(range(n_cores))],
                    ins=[bounce_in[:].opt()],
                    outs=[bounce_out[:].opt()])
            else:
                nc.sync.dma_start(bounce_out[:], bounce_in[:])
            for g in range(G):
                nc.sync.dma_start(
                    red_sb[:, g * (D + 1):(g + 1) * (D + 1)],
                    bounce_out[g, :, :])
                cnt = red_sb[:, g * (D + 1) + D:g * (D + 1) + D + 1]
                nc.vector.tensor_copy(cnt_all[:, g:g + 1], cnt)
                nc.vector.tensor_scalar_max(dv[:, g:g + 1], cnt, 1.0)
                nc.vector.reciprocal(dv[:, g:g + 1], dv[:, g:g + 1])
                nc.vector.tensor_scalar_mul(
                    newc[:, g * D:(g + 1) * D],
                    red_sb[:, g * (D + 1):g * (D + 1) + D],
                    dv[:, g:g + 1])
            nc.vector.tensor_reduce(minc[:, :], cnt_all[:, :], axis=AX.X,
                                    op=ALU.min)
            nc.vector.tensor_scalar(out=alive[:, :], in0=minc[:, :],
                                    scalar1=0.0, scalar2=None, op0=ALU.is_gt)
            for g in range(G):
                nc.vector.tensor_scalar_mul(newc[:, g * D:(g + 1) * D],
                                            newc[:, g * D:(g + 1) * D],
                                            alive[:, 0:1])
            if ep < epochs - 1:
                rebuild_ct()

        for g in range(G):
            nc.sync.dma_start(out[g, :, :], newc[:, g * D:(g + 1) * D])

    nc.compile()
    return nc


def shard_inputs(patches: np.ndarray, n_cores: int):
    """Full patches [N, G, C, H, W] -> per-core {pf_dn, pf_nd} arrays."""
    N = patches.shape[0]
    n_local = N // n_cores
    pf = np.ascontiguousarray(patches.reshape(N, G, D)).astype(np.float32,
                                                               copy=False)
    maps = []
    for c in range(n_cores):
        s = pf[c * n_local:(c + 1) * n_local]  # [n_local, G, D]
        import ml_dtypes
        bf = ml_dtypes.bfloat16
        aug = np.zeros((n_local, G, 2, NSUM), dtype=bf)
        hi = s.astype(bf)
        lo = (s - hi.astype(np.float32)).astype(bf)
        aug[:, :, 0, :D] = hi
        aug[:, :, 1, :D] = lo
        aug[:, :, 0, D:] = bf(1.0)
        maps.append({
            "pf_dn": np.ascontiguousarray(s.transpose(1, 2, 0)),
            "pf_nd": aug.reshape(n_local, G * 2 * NSUM),
        })
    return maps


_CACHE = {}


def kernel(patches: np.ndarray, centroids_init: np.ndarray) -> np.ndarray:
    patches = np.asarray(patches, dtype=np.float32)
    centroids_init = np.asarray(centroids_init, dtype=np.float32)
    N = patches.shape[0]
    n_cores = 8
    epochs = 10
    n_local = N // n_cores
    assert N % n_cores == 0

    key = (N, epochs, n_cores)
    if key not in _CACHE:
        _CACHE[key] = build_nc(n_local, epochs, n_cores)
    nc = _CACHE[key]

    in_maps = shard_inputs(patches, n_cores)
    for m in in_maps:
        m["cent"] = centroids_init

    res = bass_utils.run_bass_kernel_spmd(nc, in_maps,
                                          core_ids=list(range(n_cores)))
    c = res.results[0]["out"]
    C, H, W = 16, 5, 5
    return c.reshape(G * K, C, H, W).astype(np.float32)


if __name__ == "__main__":
    np.random.seed(0)
    p = np.random.randn(2000, G, 16, 5, 5).astype(np.float32)
    ci = (np.random.randn(G, K, D) * 0.1).astype(np.float32)
    print(kernel(p, ci).shape)
